# revision 8
# baseline (speedup 1.0000x reference)
"""KernelNorm2d Trainium2 Bass kernel (fp16 I/O, window-major layout).

Problem: x [16, 64, 256, 256] f32. 2x2 windows (stride 2) over (H, W); per-window
statistics over (C, 2, 2) = 256 elements; out = (x - mean) / sqrt(var + eps).
Data-parallel over batch: 8 cores x 2 samples each.

Host relayouts x to window-major [B, nH, nW, (c a b)] fp16, so each window's 256
elements are contiguous in SBUF (partition = window row i). All on-chip passes
are then contiguous-AP ops, and DMA runs are 32 KiB per partition.

Stats are ONE DVE pass via bn_stats (count/mean/M2 for even/odd halves of each
window pair), combined into mean/var with cheap batched vector math (the even/odd
groups have equal count=128, so var = mean(var_g) + var(mean_g)). This replaces
the baseline's two reduce passes + ACT squares. Normalize is per-window-column
scale+bias, split across ACT/GPSIMD (DVE joins only in the pipeline tail).
"""

import os
import sys

for _p in ("/opt/trn_rl_repo", "/root/.axon_site/_ro/trn_rl_repo"):
    if os.path.isdir(_p) and _p not in sys.path:
        sys.path.append(_p)

import numpy as np

import concourse.bass as bass
import concourse.tile as tile
from concourse import bacc, mybir
from concourse.bass_utils import run_bass_kernel_spmd

# Problem constants (hardcoded per spec nn_KernelNorm2d_72164040507639)
B, C, H, W = 16, 64, 256, 256
N_CORES = 8
B_LOC = B // N_CORES          # samples per core
NH = H // 2                   # 128 window rows = partition dim
NJ = W // 2                   # 128 window cols
WIN = C * 4                   # 256 elements per window
EPS = 1e-5
JH = NJ // 2                  # window cols per half (stats/normalize unit)

# normalize engine split per 64-j half (v=DVE, s=ACT, g=GPSIMD)
NV, NS = 0, 37                # steady state: DVE is stats-only
TV, TS = 24, 22               # tail: spread across all three engines


def _make_pattern(nv, ns, n=JH):
    w = {"v": nv, "s": ns, "g": n - nv - ns}
    acc = {"v": 0.0, "s": 0.0, "g": 0.0}
    pat = []
    for k in range(n):
        best = max(w, key=lambda e: w[e] / n * (k + 1) - acc[e])
        acc[best] += 1
        pat.append(best)
    return "".join(pat)


NORM_PATTERN = _make_pattern(NV, NS)
TAIL_PATTERN = _make_pattern(TV, TS)


def build_kernel(debug: bool = False) -> bass.Bass:
    nc = bacc.Bacc("TRN2", debug=debug)
    f16 = mybir.dt.float16
    f32 = mybir.dt.float32
    x = nc.dram_tensor("x", [B_LOC, NH, NJ, WIN], f16, kind="ExternalInput")
    y = nc.dram_tensor("y", [B_LOC, NH, NJ, WIN], f16, kind="ExternalOutput")

    with tile.TileContext(nc) as tc:
        with (
            tc.tile_pool(name="data", bufs=2) as data_pool,
            tc.tile_pool(name="stats", bufs=2) as stats_pool,
            tc.tile_pool(name="singles", bufs=1) as singles,
        ):
            eps_tile = singles.tile([NH, 1], f32)
            nc.vector.memset(eps_tile, EPS)

            state = {}

            def load(b, xt):
                """two j-half DMAs; 32 KiB contiguous per partition each."""
                for h in range(2):
                    js = h * JH
                    nc.sync.dma_start(
                        out=xt[:, js : js + JH], in_=x[b, :, js : js + JH]
                    )

            def stats(b, h, xt):
                """one bn_stats pass + combine for one j-half (JH cols)."""
                js = h * JH
                S = stats_pool.tile([NH, JH, 2, 3], f16, tag=f"S{h}")
                for t in range(JH):
                    nc.vector.bn_stats(out=S[:, t], in_=xt[:, js + t, :])
                # per-window mean/var from the two equal count=128 groups:
                # mu = Sm/2; var = Sv/256 + Sq/2 - mu^2
                m_view = S[:, :, :, 1]
                v_view = S[:, :, :, 2]
                msq = stats_pool.tile([NH, JH, 2], f32, tag=f"msq{h}")
                sm = stats_pool.tile([NH, JH], f32, tag=f"sm{h}")
                sq = stats_pool.tile([NH, JH], f32, tag=f"sq{h}")
                sv = stats_pool.tile([NH, JH], f32, tag=f"sv{h}")
                nm = stats_pool.tile([NH, JH], f32, tag=f"nm{h}")
                var = stats_pool.tile([NH, JH], f32, tag=f"var{h}")
                istd = stats_pool.tile([NH, JH], f32, tag=f"istd{h}")
                tsh = stats_pool.tile([NH, JH], f32, tag=f"tsh{h}")
                nc.scalar.activation(
                    out=msq, in_=m_view, func=mybir.ActivationFunctionType.Square
                )
                nc.vector.tensor_reduce(
                    out=sm, in_=m_view, axis=mybir.AxisListType.X,
                    op=mybir.AluOpType.add,
                )
                nc.vector.tensor_reduce(
                    out=sq, in_=msq, axis=mybir.AxisListType.X,
                    op=mybir.AluOpType.add,
                )
                nc.vector.tensor_reduce(
                    out=sv, in_=v_view, axis=mybir.AxisListType.X,
                    op=mybir.AluOpType.add,
                )
                nc.vector.tensor_scalar_mul(out=nm, in0=sm, scalar1=-0.5)
                # var = sv/256 + sq/2 - nm*nm
                nc.vector.tensor_scalar_mul(out=var, in0=sv, scalar1=1.0 / WIN)
                nc.vector.tensor_scalar_mul(out=sq, in0=sq, scalar1=0.5)
                nc.vector.tensor_add(out=var, in0=var, in1=sq)
                nc.vector.tensor_mul(out=sq, in0=nm, in1=nm)
                nc.vector.tensor_tensor(
                    out=var, in0=var, in1=sq, op=mybir.AluOpType.subtract
                )
                nc.scalar.activation(
                    out=var, in_=var, func=mybir.ActivationFunctionType.Sqrt,
                    bias=eps_tile, scale=1.0,
                )
                nc.vector.reciprocal(out=istd, in_=var)
                nc.vector.tensor_mul(out=tsh, in0=nm, in1=istd)
                state[(b, h)] = (xt, istd, tsh, nm)

            def normalize(b, h, pattern=NORM_PATTERN):
                """normalize half in place, then store it.

                pattern "B" = two-pass broadcast tensor_tensor on GPSIMD;
                otherwise per-column with engine chosen by pattern char."""
                xt, istd, tsh, nm = state.pop((b, h))
                js = h * JH
                xh = xt[:, js : js + JH]
                if pattern == "B":
                    nc.gpsimd.tensor_tensor(
                        out=xh,
                        in0=xh,
                        in1=nm.to_broadcast([NH, JH, WIN]),
                        op=mybir.AluOpType.add,
                    )
                    nc.gpsimd.tensor_tensor(
                        out=xh,
                        in0=xh,
                        in1=istd.to_broadcast([NH, JH, WIN]),
                        op=mybir.AluOpType.mult,
                    )
                else:
                    for jo in range(JH):
                        win = xt[:, js + jo, :]
                        eng = pattern[jo]
                        if eng == "s":
                            nc.scalar.activation(
                                out=win,
                                in_=win,
                                func=mybir.ActivationFunctionType.Identity,
                                bias=tsh[:, jo : jo + 1],
                                scale=istd[:, jo : jo + 1],
                            )
                        else:
                            e = nc.vector if eng == "v" else nc.gpsimd
                            e.tensor_scalar(
                                out=win,
                                in0=win,
                                scalar1=istd[:, jo : jo + 1],
                                scalar2=tsh[:, jo : jo + 1],
                                op0=mybir.AluOpType.mult,
                                op1=mybir.AluOpType.add,
                            )
                nc.scalar.dma_start(
                    out=y[b, :, js : js + JH], in_=xt[:, js : js + JH]
                )

            # software-pipelined emission over (sample, j-half) units
            xt0 = data_pool.tile([NH, NJ, WIN], f16, tag="xt")
            xt1 = data_pool.tile([NH, NJ, WIN], f16, tag="xt")
            # A/B probes this round: (0,0) GPSIMD broadcast, (0,1) all-DVE
            # cols, (1,0) all-ACT cols, (1,1) mixed tail
            load(0, xt0)
            load(1, xt1)
            stats(0, 0, xt0)
            normalize(0, 0, "B")
            stats(0, 1, xt0)
            normalize(0, 1, "v" * JH)
            stats(1, 0, xt1)
            normalize(1, 0, "s" * JH)
            stats(1, 1, xt1)
            normalize(1, 1, TAIL_PATTERN)
    nc.compile()
    return nc


_NC_CACHE = None
LAST_RESULTS = None


def _get_nc():
    global _NC_CACHE
    if _NC_CACHE is None:
        _NC_CACHE = build_kernel()
    return _NC_CACHE


def kernel(x: np.ndarray) -> np.ndarray:
    global LAST_RESULTS
    assert x.shape == (B, C, H, W), x.shape
    # window-major host relayout: [B, C, H, W] -> [B, nH, nW, (c a b)] fp16
    xh = np.ascontiguousarray(
        x.astype(np.float16)
        .reshape(B, C, NH, 2, NJ, 2)
        .transpose(0, 2, 4, 1, 3, 5)
        .reshape(B, NH, NJ, WIN)
    )
    nc = _get_nc()
    in_maps = [{"x": xh[k * B_LOC : (k + 1) * B_LOC]} for k in range(N_CORES)]
    kw = {}
    if os.environ.get("KERNEL_TRACE") == "1":
        kw["trace"] = True
        if os.environ.get("KERNEL_TRACE_DIR"):
            import tempfile

            base = os.environ["KERNEL_TRACE_DIR"]
            os.makedirs(base, exist_ok=True)
            kw["tmpdir"] = tempfile.mkdtemp(dir=base)
    res = run_bass_kernel_spmd(nc, in_maps, core_ids=list(range(N_CORES)), **kw)
    LAST_RESULTS = res
    out = np.concatenate([r["y"] for r in res.results], axis=0)
    return (
        out.reshape(B, NH, NJ, C, 2, 2)
        .transpose(0, 3, 1, 4, 2, 5)
        .reshape(B, C, H, W)
        .astype(np.float32)
    )


# revision 9
# speedup vs baseline: 1.8081x; 1.8081x over previous
"""KernelNorm2d Trainium2 Bass kernel (fp16 I/O, window-major layout).

Problem: x [16, 64, 256, 256] f32. 2x2 windows (stride 2) over (H, W); per-window
statistics over (C, 2, 2) = 256 elements; out = (x - mean) / sqrt(var + eps).
Data-parallel over batch: 8 cores x 2 samples each.

Host relayouts x to window-major [B, nH, nW, (c a b)] fp16, so each window's 256
elements are contiguous in SBUF (partition = window row i). DMA runs are 32 KiB
per partition.

Stats are ONE DVE pass via bn_stats (one instr per window: 6-tuple of
count/mean/count*var for the even/odd halves, i.e. two equal groups of 128).
Pooling the two groups: mu = (m0+m1)/2, var = (cv0+cv1 + 128*(m0^2+m1^2)
- 64*(m0+m1)^2) / 256, computed with batched DVE ops + one fused ACT
sqrt(x*scale + eps). Normalize is per-window-column scale+bias split across
ACT/GPSIMD in steady state (DVE is stats-bound); the pipeline head and tail run
at quarter granularity with DVE joining the tail.

Measured instruction facts (HW traces): bn_stats 327 ns/window (no DVE 2x mode
exists for it); per-col normalize: DVE 305 ns, ACT 495 ns, GPSIMD 537 ns; GPSIMD
big tensor_tensor 2 ns/elem (useless); ACT big ops ~0.8 ns/elem + ~300 ns fixed.
"""

import os
import sys

for _p in ("/opt/trn_rl_repo", "/root/.axon_site/_ro/trn_rl_repo"):
    if os.path.isdir(_p) and _p not in sys.path:
        sys.path.append(_p)

import numpy as np

import concourse.bass as bass
import concourse.tile as tile
from concourse import bacc, mybir
from concourse.bass_utils import run_bass_kernel_spmd

# Problem constants (hardcoded per spec nn_KernelNorm2d_72164040507639)
B, C, H, W = 16, 64, 256, 256
N_CORES = 8
B_LOC = B // N_CORES          # samples per core
NH = H // 2                   # 128 window rows = partition dim
NJ = W // 2                   # 128 window cols
WIN = C * 4                   # 256 elements per window
EPS = 1e-5


def _make_pattern(nv, ns, n):
    w = {"v": nv, "s": ns, "g": n - nv - ns}
    acc = {"v": 0.0, "s": 0.0, "g": 0.0}
    pat = []
    for k in range(n):
        best = max(w, key=lambda e: w[e] / n * (k + 1) - acc[e])
        acc[best] += 1
        pat.append(best)
    return "".join(pat)


P64 = _make_pattern(0, 33, 64)    # steady half: ACT/GPSIMD only
P32 = _make_pattern(0, 17, 32)    # steady quarter
P32T = _make_pattern(13, 10, 32)  # tail quarter: all three engines


def build_kernel(debug: bool = False) -> bass.Bass:
    nc = bacc.Bacc("TRN2", debug=debug)
    f16 = mybir.dt.float16
    f32 = mybir.dt.float32
    x = nc.dram_tensor("x", [B_LOC, NH, NJ, WIN], f16, kind="ExternalInput")
    y = nc.dram_tensor("y", [B_LOC, NH, NJ, WIN], f16, kind="ExternalOutput")

    with tile.TileContext(nc) as tc:
        with (
            tc.tile_pool(name="data", bufs=2) as data_pool,
            tc.tile_pool(name="stats", bufs=2) as stats_pool,
            tc.tile_pool(name="singles", bufs=1) as singles,
        ):
            eps_tile = singles.tile([NH, 1], f32)
            nc.vector.memset(eps_tile, EPS)

            state = {}

            def stats(b, js, jn, xt):
                """bn_stats per window + pooled mean/var for cols [js, js+jn)."""
                tg = f"{js}_{jn}"
                S = stats_pool.tile([NH, jn, 2, 3], f32, tag=f"S{tg}")
                for t in range(jn):
                    nc.vector.bn_stats(out=S[:, t], in_=xt[:, js + t, :])
                m_view = S[:, :, :, 1]
                v_view = S[:, :, :, 2]
                sm = stats_pool.tile([NH, jn], f32, tag=f"sm{tg}")
                sq = stats_pool.tile([NH, jn], f32, tag=f"sq{tg}")
                sv = stats_pool.tile([NH, jn], f32, tag=f"sv{tg}")
                msq = stats_pool.tile([NH, jn, 2], f32, tag=f"msq{tg}")
                nm = stats_pool.tile([NH, jn], f32, tag=f"nm{tg}")
                var = stats_pool.tile([NH, jn], f32, tag=f"var{tg}")
                istd = stats_pool.tile([NH, jn], f32, tag=f"istd{tg}")
                tsh = stats_pool.tile([NH, jn], f32, tag=f"tsh{tg}")
                nc.vector.tensor_mul(out=msq, in0=m_view, in1=m_view)
                nc.vector.tensor_reduce(
                    out=sm, in_=m_view, axis=mybir.AxisListType.X,
                    op=mybir.AluOpType.add,
                )
                nc.vector.tensor_reduce(
                    out=sq, in_=msq, axis=mybir.AxisListType.X,
                    op=mybir.AluOpType.add,
                )
                nc.vector.tensor_reduce(
                    out=sv, in_=v_view, axis=mybir.AxisListType.X,
                    op=mybir.AluOpType.add,
                )
                # 256*var = sv + 128*sq - 64*sm^2 ; istd = rsqrt(var + eps)
                nc.vector.tensor_scalar_mul(out=nm, in0=sm, scalar1=-0.5)
                nc.vector.scalar_tensor_tensor(
                    out=var, in0=sq, scalar=128.0, in1=sv,
                    op0=mybir.AluOpType.mult, op1=mybir.AluOpType.add,
                )
                nc.vector.tensor_mul(out=sq, in0=sm, in1=sm)
                nc.vector.scalar_tensor_tensor(
                    out=var, in0=sq, scalar=-64.0, in1=var,
                    op0=mybir.AluOpType.mult, op1=mybir.AluOpType.add,
                )
                nc.scalar.activation(
                    out=var, in_=var, func=mybir.ActivationFunctionType.Sqrt,
                    bias=eps_tile, scale=1.0 / WIN,
                )
                nc.vector.reciprocal(out=istd, in_=var)
                nc.vector.tensor_mul(out=tsh, in0=nm, in1=istd)
                state[(b, js)] = (xt, istd, tsh)

            def normalize(b, js, jn, pattern):
                """normalize cols [js, js+jn) in place, then store them."""
                xt, istd, tsh = state.pop((b, js))
                for jo in range(jn):
                    win = xt[:, js + jo, :]
                    eng = pattern[jo]
                    if eng == "s":
                        nc.scalar.activation(
                            out=win,
                            in_=win,
                            func=mybir.ActivationFunctionType.Identity,
                            bias=tsh[:, jo : jo + 1],
                            scale=istd[:, jo : jo + 1],
                        )
                    else:
                        e = nc.vector if eng == "v" else nc.gpsimd
                        e.tensor_scalar(
                            out=win,
                            in0=win,
                            scalar1=istd[:, jo : jo + 1],
                            scalar2=tsh[:, jo : jo + 1],
                            op0=mybir.AluOpType.mult,
                            op1=mybir.AluOpType.add,
                        )
                nc.scalar.dma_start(
                    out=y[b, :, js : js + jn], in_=xt[:, js : js + jn]
                )

            # software-pipelined units; quarter-sized at the head (early DVE
            # start after a 2.1 MB load) and tail (short drain)
            xt0 = data_pool.tile([NH, NJ, WIN], f16, tag="xt")
            xt1 = data_pool.tile([NH, NJ, WIN], f16, tag="xt")
            for js, jn in ((0, 32), (32, 32), (64, 64)):
                nc.sync.dma_start(
                    out=xt0[:, js : js + jn], in_=x[0, :, js : js + jn]
                )
            for js, jn in ((0, 64), (64, 64)):
                nc.sync.dma_start(
                    out=xt1[:, js : js + jn], in_=x[1, :, js : js + jn]
                )
            units = [
                (0, 0, 32, P32),
                (0, 32, 32, P32),
                (0, 64, 64, P64),
                (1, 0, 64, P64),
                (1, 64, 32, P32),
                (1, 96, 32, P32T),
            ]
            for b, js, jn, pat in units:
                stats(b, js, jn, xt0 if b == 0 else xt1)
                normalize(b, js, jn, pat)
    nc.compile()
    return nc


_NC_CACHE = None
LAST_RESULTS = None


def _get_nc():
    global _NC_CACHE
    if _NC_CACHE is None:
        _NC_CACHE = build_kernel()
    return _NC_CACHE


def kernel(x: np.ndarray) -> np.ndarray:
    global LAST_RESULTS
    assert x.shape == (B, C, H, W), x.shape
    # window-major host relayout: [B, C, H, W] -> [B, nH, nW, (c a b)] fp16
    xh = np.ascontiguousarray(
        x.astype(np.float16)
        .reshape(B, C, NH, 2, NJ, 2)
        .transpose(0, 2, 4, 1, 3, 5)
        .reshape(B, NH, NJ, WIN)
    )
    nc = _get_nc()
    in_maps = [{"x": xh[k * B_LOC : (k + 1) * B_LOC]} for k in range(N_CORES)]
    kw = {}
    if os.environ.get("KERNEL_TRACE") == "1":
        kw["trace"] = True
        if os.environ.get("KERNEL_TRACE_DIR"):
            import tempfile

            base = os.environ["KERNEL_TRACE_DIR"]
            os.makedirs(base, exist_ok=True)
            kw["tmpdir"] = tempfile.mkdtemp(dir=base)
    res = run_bass_kernel_spmd(nc, in_maps, core_ids=list(range(N_CORES)), **kw)
    LAST_RESULTS = res
    out = np.concatenate([r["y"] for r in res.results], axis=0)
    return (
        out.reshape(B, NH, NJ, C, 2, 2)
        .transpose(0, 3, 1, 4, 2, 5)
        .reshape(B, C, H, W)
        .astype(np.float32)
    )


# revision 15
# speedup vs baseline: 1.8238x; 1.0087x over previous
"""KernelNorm2d Trainium2 Bass kernel (fp16 I/O, window-major layout).

Problem: x [16, 64, 256, 256] f32. 2x2 windows (stride 2) over (H, W); per-window
statistics over (C, 2, 2) = 256 elements; out = (x - mean) / sqrt(var + eps).
Data-parallel over batch: 8 cores x 2 samples each.

Host relayouts x to window-major [B, nH, nW, (c a b)] fp16, so each window's 256
elements are contiguous in SBUF (partition = window row i). DMA runs are 32 KiB
per partition.

Stats are ONE DVE pass via bn_stats (one instr per window: 6-tuple of
count/mean/count*var for the even/odd halves, i.e. two equal groups of 128).
Pooling the two groups: mu = (m0+m1)/2, var = (cv0+cv1 + 128*(m0^2+m1^2)
- 64*(m0+m1)^2) / 256, computed with batched DVE ops + one fused ACT
sqrt(x*scale + eps). Normalize is per-window-column scale+bias split across
ACT/GPSIMD in steady state (DVE is stats-bound); the pipeline head and tail run
at quarter granularity with DVE joining the tail.

Measured instruction facts (HW traces): bn_stats 327 ns/window (no DVE 2x mode
exists for it); per-col normalize: DVE 305 ns, ACT 495 ns, GPSIMD 537 ns; GPSIMD
big tensor_tensor 2 ns/elem (useless); ACT big ops ~0.8 ns/elem + ~300 ns fixed.
"""

import os
import sys

for _p in ("/opt/trn_rl_repo", "/root/.axon_site/_ro/trn_rl_repo"):
    if os.path.isdir(_p) and _p not in sys.path:
        sys.path.append(_p)

import numpy as np

import concourse.bass as bass
import concourse.tile as tile
from concourse import bacc, mybir
from concourse.bass_utils import run_bass_kernel_spmd

# Problem constants (hardcoded per spec nn_KernelNorm2d_72164040507639)
B, C, H, W = 16, 64, 256, 256
N_CORES = 8
B_LOC = B // N_CORES          # samples per core
NH = H // 2                   # 128 window rows = partition dim
NJ = W // 2                   # 128 window cols
WIN = C * 4                   # 256 elements per window
EPS = 1e-5


def _make_pattern(nv, ns, n):
    w = {"v": nv, "s": ns, "g": n - nv - ns}
    acc = {"v": 0.0, "s": 0.0, "g": 0.0}
    pat = []
    for k in range(n):
        best = max(w, key=lambda e: w[e] / n * (k + 1) - acc[e])
        acc[best] += 1
        pat.append(best)
    return "".join(pat)


P64 = _make_pattern(0, 33, 64)    # steady half: ACT/GPSIMD only
P32 = _make_pattern(0, 17, 32)    # steady quarter
P16 = _make_pattern(0, 9, 16)     # steady eighth
P16T = _make_pattern(6, 5, 16)    # tail eighth: all three engines


def build_kernel(debug: bool = False) -> bass.Bass:
    nc = bacc.Bacc("TRN2", debug=debug)
    f16 = mybir.dt.float16
    f32 = mybir.dt.float32
    x = nc.dram_tensor("x", [B_LOC, NH, NJ, WIN], f16, kind="ExternalInput")
    y = nc.dram_tensor("y", [B_LOC, NH, NJ, WIN], f16, kind="ExternalOutput")

    with tile.TileContext(nc) as tc:
        with (
            tc.tile_pool(name="data", bufs=2) as data_pool,
            tc.tile_pool(name="stats", bufs=2) as stats_pool,
            tc.tile_pool(name="singles", bufs=1) as singles,
        ):
            eps_tile = singles.tile([NH, 1], f32)
            nc.vector.memset(eps_tile, EPS)

            state = {}

            def stats(b, js, jn, xt):
                """bn_stats per window + pooled mean/var for cols [js, js+jn)."""
                tg = f"{js}_{jn}"
                S = stats_pool.tile([NH, jn, 2, 3], f32, tag=f"S{tg}")
                for t in range(jn):
                    nc.vector.bn_stats(out=S[:, t], in_=xt[:, js + t, :])
                m_view = S[:, :, :, 1]
                v_view = S[:, :, :, 2]
                sm = stats_pool.tile([NH, jn], f32, tag=f"sm{tg}")
                sq = stats_pool.tile([NH, jn], f32, tag=f"sq{tg}")
                sv = stats_pool.tile([NH, jn], f32, tag=f"sv{tg}")
                msq = stats_pool.tile([NH, jn, 2], f32, tag=f"msq{tg}")
                var = stats_pool.tile([NH, jn], f32, tag=f"var{tg}")
                istd = stats_pool.tile([NH, jn], f32, tag=f"istd{tg}")
                tsh = stats_pool.tile([NH, jn], f32, tag=f"tsh{tg}")
                nc.vector.tensor_mul(out=msq, in0=m_view, in1=m_view)
                nc.vector.tensor_reduce(
                    out=sm, in_=m_view, axis=mybir.AxisListType.X,
                    op=mybir.AluOpType.add,
                )
                nc.vector.tensor_reduce(
                    out=sq, in_=msq, axis=mybir.AxisListType.X,
                    op=mybir.AluOpType.add,
                )
                nc.vector.tensor_reduce(
                    out=sv, in_=v_view, axis=mybir.AxisListType.X,
                    op=mybir.AluOpType.add,
                )
                # 256*var = sv + 128*sq - 64*sm^2 ; istd = rsqrt(var + eps)
                nc.vector.scalar_tensor_tensor(
                    out=var, in0=sq, scalar=128.0, in1=sv,
                    op0=mybir.AluOpType.mult, op1=mybir.AluOpType.add,
                )
                nc.vector.tensor_mul(out=sq, in0=sm, in1=sm)
                nc.vector.scalar_tensor_tensor(
                    out=var, in0=sq, scalar=-64.0, in1=var,
                    op0=mybir.AluOpType.mult, op1=mybir.AluOpType.add,
                )
                nc.scalar.activation(
                    out=var, in_=var, func=mybir.ActivationFunctionType.Sqrt,
                    bias=eps_tile, scale=1.0 / WIN,
                )
                nc.vector.reciprocal(out=istd, in_=var)
                # tsh = -mu * istd = (sm * -0.5) * istd
                nc.vector.scalar_tensor_tensor(
                    out=tsh, in0=sm, scalar=-0.5, in1=istd,
                    op0=mybir.AluOpType.mult, op1=mybir.AluOpType.mult,
                )
                state[(b, js)] = (xt, istd, tsh)

            def normalize(b, js, jn, pattern, store_q):
                """normalize cols [js, js+jn) in place, then store them."""
                xt, istd, tsh = state.pop((b, js))
                for jo in range(jn):
                    win = xt[:, js + jo, :]
                    eng = pattern[jo]
                    if eng == "s":
                        nc.scalar.activation(
                            out=win,
                            in_=win,
                            func=mybir.ActivationFunctionType.Identity,
                            bias=tsh[:, jo : jo + 1],
                            scale=istd[:, jo : jo + 1],
                        )
                    else:
                        e = nc.vector if eng == "v" else nc.gpsimd
                        e.tensor_scalar(
                            out=win,
                            in0=win,
                            scalar1=istd[:, jo : jo + 1],
                            scalar2=tsh[:, jo : jo + 1],
                            op0=mybir.AluOpType.mult,
                            op1=mybir.AluOpType.add,
                        )
                store_q.dma_start(
                    out=y[b, :, js : js + jn], in_=xt[:, js : js + jn]
                )

            # software-pipelined units; eighth-sized at the head (early DVE
            # start after a 1 MB load) and tail (short drain). Two HW DGE
            # rings (sync, scalar): loads on sync; stores on scalar, except
            # late stores that reuse the by-then-idle sync ring.
            xt0 = data_pool.tile([NH, NJ, WIN], f16, tag="xt")
            xt1 = data_pool.tile([NH, NJ, WIN], f16, tag="xt")
            for xt, b, js, jn in (
                (xt0, 0, 0, 16),
                (xt0, 0, 16, 16),
                (xt0, 0, 32, 32),
                (xt0, 0, 64, 64),
                (xt1, 1, 0, 64),
                (xt1, 1, 64, 64),
            ):
                nc.sync.dma_start(
                    out=xt[:, js : js + jn], in_=x[b, :, js : js + jn]
                )
            units = [
                (0, 0, 16, P16, nc.scalar),
                (0, 16, 16, P16, nc.scalar),
                (0, 32, 32, P32, nc.scalar),
                (0, 64, 64, P64, nc.scalar),
                (1, 0, 64, P64, nc.sync),
                (1, 64, 32, P32, nc.scalar),
                (1, 96, 16, P16, nc.scalar),
                (1, 112, 16, P16T, nc.sync),
            ]
            for b, js, jn, pat, q in units:
                stats(b, js, jn, xt0 if b == 0 else xt1)
                normalize(b, js, jn, pat, q)
    nc.compile()
    return nc


_NC_CACHE = None
LAST_RESULTS = None


def _get_nc():
    global _NC_CACHE
    if _NC_CACHE is None:
        _NC_CACHE = build_kernel()
    return _NC_CACHE


def kernel(x: np.ndarray) -> np.ndarray:
    global LAST_RESULTS
    assert x.shape == (B, C, H, W), x.shape
    # window-major host relayout: [B, C, H, W] -> [B, nH, nW, (c a b)] fp16
    xh = np.ascontiguousarray(
        x.astype(np.float16)
        .reshape(B, C, NH, 2, NJ, 2)
        .transpose(0, 2, 4, 1, 3, 5)
        .reshape(B, NH, NJ, WIN)
    )
    nc = _get_nc()
    in_maps = [{"x": xh[k * B_LOC : (k + 1) * B_LOC]} for k in range(N_CORES)]
    kw = {}
    if os.environ.get("KERNEL_TRACE") == "1":
        kw["trace"] = True
        if os.environ.get("KERNEL_TRACE_DIR"):
            import tempfile

            base = os.environ["KERNEL_TRACE_DIR"]
            os.makedirs(base, exist_ok=True)
            kw["tmpdir"] = tempfile.mkdtemp(dir=base)
    res = run_bass_kernel_spmd(nc, in_maps, core_ids=list(range(N_CORES)), **kw)
    LAST_RESULTS = res
    out = np.concatenate([r["y"] for r in res.results], axis=0)
    return (
        out.reshape(B, NH, NJ, C, 2, 2)
        .transpose(0, 3, 1, 4, 2, 5)
        .reshape(B, C, H, W)
        .astype(np.float32)
    )
